# revision 37
# baseline (speedup 1.0000x reference)
"""Trainium2 Bass kernel for nn_Causal_Attention_13082470383895.

Full (unsharded) inputs in, full output out. Internally shards batch*heads
across 8 NeuronCores: core c owns batch c//4 and the 4 heads [4*(c%4), 4*(c%4)+4).
Each core computes its heads' q/k/v projections (column-sharded weights),
QK-layernorm, causal unnormalized-exp attention, and its partial contribution
to the output projection (row-sharded W_out). Host sums the 4 partials per batch.

Perf notes vs the first working version (414us -> 232us):
- x is transposed on the host, so the kernel DMAs x^T directly and skips the
  128 PE transposes + their PSUM evacuation copies per core.
- everything on the PE runs in bf16 (host-cast); PSUM accumulation stays f32.
- layernorm runs as: bf16 stats (tensor_reduce of qk and qk^2), batched
  Ln/Exp rstd on ScalarE, then two broadcast tensor_tensors per tile on DVE
  (the original per-group GpSimd tensor_scalar path measured ~1.25us/op).
- exp for a head-pair is one ACTIVATE over a 2-bank PSUM tile [128,2,512]
  (halves the per-instruction 352-cycle fixed cost); exp is the dominant
  ScalarE load (~82us) and paces the attention inner loop.
- softmax denominators for a head-pair land on one 32-aligned partition
  (free slot = r01); one reciprocal_approx_fast + f32r cast per supertile,
  then a K=1 PE matmul broadcasts 1/den across 64 partitions per head.
- the av matmul trails its scores matmul by 4 iterations so the
  st -> exp -> mask -> av cross-engine latency is hidden.
- emission zippers projection(s+1), finalize(s-1) and out-projection units
  into attention(s)'s j-loop (front-loaded A-units, out-projections lag into
  supertile 3's long thin loop) to keep the PE dense enough that the HAM
  clock-gate mostly stays at 2.4 GHz.
"""

import os
import sys

import numpy as np

sys.path.insert(0, "/opt/trn_rl_repo")

B = 2
L = 2048
D = 1024
HEADS = 16
DIM = 64
LN_EPS = 1e-6
P = 128
LT = L // P          # 16 l-tiles
DT = D // P          # 8 contraction tiles
NHL = 4              # heads per core
SUP = 4              # 512-wide l supertiles
N_CORES = 8
EPS_RAW = float(D * LN_EPS)  # LN eps folded for raw (unscaled) qk

_CACHE = {}


def _make_bacc_cls():
    import bass_rust
    import concourse.mybir as mybir
    from concourse import bacc
    from concourse.hw_specs import get_activation_tables

    class KernelBacc(bacc.Bacc):
        """Bacc whose ACT-table selector never picks the `natural_log` set
        for Ln: hiding `ln` there makes the greedy selector choose
        `natural_log_exp_and_others` (which also holds exp/copy), so the
        kernel needs a single table load instead of thrashing
        exp_and_others <-> natural_log on every layernorm."""

        def insert_act_table_loads(self):
            has_activation = any(
                isinstance(i, mybir.InstActivation)
                for b in self.main_func.blocks
                for i in b.instructions
            )
            if not has_activation:
                return
            ln = mybir.ActivationFunctionType.Ln
            tables = []
            for name, funcs in get_activation_tables(self.m.arch).items():
                if name == "natural_log":
                    funcs = funcs - {ln}
                tables.append((name, funcs))
            bass_rust.insert_act_table_loads(self, tables)

    return KernelBacc


def _build_nc():
    import concourse.bass as bass  # noqa: F401
    import concourse.mybir as mybir
    import concourse.tile as tile
    from concourse.masks import make_identity, make_upper_triangular

    f32 = mybir.dt.float32
    f32r = mybir.dt.float32r
    bf16 = mybir.dt.bfloat16
    AF = mybir.ActivationFunctionType
    ALU = mybir.AluOpType

    nc = _make_bacc_cls()("TRN2", target_bir_lowering=False, debug=False)

    XT = nc.dram_tensor("xt", [SUP, D, 512], bf16, kind="ExternalInput").ap()
    WQK = nc.dram_tensor("w_qk", [D, 512], bf16, kind="ExternalInput").ap()
    WV = nc.dram_tensor("w_v", [D, 256], bf16, kind="ExternalInput").ap()
    WOUT = nc.dram_tensor("w_out", [256, D], bf16, kind="ExternalInput").ap()
    OUT = nc.dram_tensor("out", [L, D], f32, kind="ExternalOutput").ap()

    with tile.TileContext(nc) as tc:
        const = tc.alloc_tile_pool(name="const", bufs=1)
        big = tc.alloc_tile_pool(name="big", bufs=1)
        work = tc.alloc_tile_pool(name="work", bufs=2)
        esp = tc.alloc_tile_pool(name="esp", bufs=4)
        outp = tc.alloc_tile_pool(name="outp", bufs=2)

        # Input DMAs are dispatched first so the transfers overlap the
        # const setup + engine-boot preamble below.
        xt = [big.tile([P, DT, 512], bf16, name=f"xt{s}") for s in range(SUP)]
        wqk = big.tile([P, DT, 512], bf16)
        wv = big.tile([P, DT, 256], bf16)
        wout = big.tile([P, 2, D], bf16)

        def dma_xt(s):
            nc.sync.dma_start(xt[s], XT[s].rearrange("(o p) l -> p o l", p=P))

        # chunk the first x^T / wqk transfers so the first projection matmul
        # is gated on ~1MB, not the full 3MB prologue
        nc.sync.dma_start(xt[0][:, :2],
                          XT[0][:256].rearrange("(o p) l -> p o l", p=P))
        nc.sync.dma_start(wqk[:, :2],
                          WQK[:256].rearrange("(o p) n -> p o n", p=P))
        nc.sync.dma_start(xt[0][:, 2:],
                          XT[0][256:].rearrange("(o p) l -> p o l", p=P))
        nc.sync.dma_start(wqk[:, 2:],
                          WQK[256:].rearrange("(o p) n -> p o n", p=P))
        nc.sync.dma_start(wv, WV.rearrange("(o p) n -> p o n", p=P))
        dma_xt(1)
        nc.sync.dma_start(wout, WOUT.rearrange("(c p) n -> p c n", p=P))

        ident = const.tile([P, P], bf16)
        make_identity(nc, ident)
        # 0/1 upper-triangular (incl diagonal) for post-exp causal masking of
        # the diagonal 128x128 block: es layout is S^T (k on partitions), so
        # valid = (q >= k) = upper triangle.
        up01 = const.tile([P, P], bf16)
        make_upper_triangular(nc, up01, val=1.0, diag=True)
        epsb = const.tile([P, 1], f32)
        nc.vector.memset(epsb, EPS_RAW)
        ones_bf = const.tile([P, 1], bf16)
        nc.vector.memset(ones_bf, 1.0)
        # stationary for the K=1 denominator-broadcast matmul; sliced at the
        # moving operand's base partition (they must match). f32r tiles can't
        # be memset directly — cast from f32 via DVE.
        ones_f32 = const.tile([P, 1], f32)
        nc.vector.memset(ones_f32, 1.0)
        ones_all = const.tile([P, DIM], f32r)
        nc.vector.tensor_copy(ones_all, ones_f32[:, 0:1].to_broadcast([P, DIM]))

        # Collapse const-setup waits behind one barrier (wait-slot limits).
        tc.strict_bb_all_engine_barrier()

        # persistent intermediates. qt/kt/at pair 2 heads on the partition
        # axis: head 2i in rows 0:64, head 2i+1 in rows 64:128.
        # v is stored augmented per head: [v_h | 1] (65 cols) so one AV
        # matmul yields both the numerator (rows 0:64) and the softmax
        # denominator (row 64).
        v_sb = big.tile([P, LT, NHL, DIM + 1], bf16)
        qt = [big.tile([P, L], bf16, name=f"qt{i}") for i in range(2)]
        kt = [big.tile([P, L], bf16, name=f"kt{i}") for i in range(2)]
        at = [big.tile([P, L], bf16, name=f"at{i}") for i in range(2)]
        nc.vector.tensor_copy(
            v_sb[:, :, :, DIM],
            ones_bf[:, 0:1].to_broadcast([P, LT, NHL]),
        )

        qk_tiles = {}   # (s, i) -> qk_sb tile
        stat_tiles = {}  # s -> per-supertile stat tile

        with tc.tile_pool(name="ps", bufs=2, space="PSUM") as ps:
            # PSUM budget (8 banks): pj 2 + st 2x[128,2,512] (4 banks) +
            # av 1x[65,2,512] (2 banks). tp/bc/op share the pj tag.

            def unit_proj(t):
                """Projection + LN stats for l-tile t."""
                s, i = t // 4, t % 4
                xts = xt[s]
                qk_ps = ps.tile([P, 512], f32, tag="pj", name="qk_ps")
                for d in range(DT):
                    nc.tensor.matmul(
                        qk_ps, xts[:, d, i * P:(i + 1) * P], wqk[:, d],
                        start=(d == 0), stop=(d == DT - 1),
                    )
                v_ps = ps.tile([P, 256], f32, tag="pj", name="v_ps")
                for d in range(DT):
                    nc.tensor.matmul(
                        v_ps, xts[:, d, i * P:(i + 1) * P], wv[:, d],
                        start=(d == 0), stop=(d == DT - 1),
                    )
                qk_sb = work.tile([P, 8, DIM], bf16, tag="qk", bufs=6,
                                  name="qk_sb")
                nc.scalar.copy(
                    qk_sb, qk_ps.rearrange("p (g d) -> p g d", g=8))
                qk_tiles[(s, i)] = qk_sb
                # v scaled by 1/sqrt(D)=1/32 here; the out-proj 1/32 is
                # folded into wout on the host.
                nc.scalar.mul(
                    v_sb[:, t, :, :DIM],
                    v_ps.rearrange("p (h d) -> p h d", h=NHL), 1.0 / 32.0)
                # LN stats over each 64-group (raw qk: eps folded as D*eps)
                if s not in stat_tiles:
                    stat_tiles[s] = work.tile([P, 4, 8, 8], bf16, tag="stat",
                                              bufs=2, name="stat_t")
                st_ = stat_tiles[s]
                sq = work.tile([P, 8, DIM], bf16, tag="sq", bufs=2,
                               name="sq_t")
                nc.vector.tensor_tensor(sq, qk_sb, qk_sb, ALU.mult)
                with nc.allow_low_precision(
                        reason="bf16 LN stats; var >> mean^2 here"):
                    nc.vector.tensor_reduce(
                        st_[:, i, :, 0], qk_sb, axis=mybir.AxisListType.X,
                        op=ALU.add)
                    nc.vector.tensor_reduce(
                        st_[:, i, :, 1], sq, axis=mybir.AxisListType.X,
                        op=ALU.add)

            def unit_ln_finish(s):
                """Batched rstd for all 4 l-tiles of supertile s, then apply."""
                st_ = stat_tiles.pop(s)
                sums = st_[:, :, :, 0]
                sumsq = st_[:, :, :, 1]
                mean = st_[:, :, :, 2]
                mn2 = st_[:, :, :, 3]
                var = st_[:, :, :, 4]
                rstd = st_[:, :, :, 5]
                prod = st_[:, :, :, 6]
                nc.vector.tensor_scalar_mul(mean, sums, 1.0 / DIM)
                nc.vector.tensor_tensor(mn2, mean, mean, ALU.mult)
                nc.vector.tensor_scalar_mul(var, sumsq, 1.0 / DIM)
                nc.vector.tensor_tensor(var, var, mn2, ALU.subtract)
                nc.scalar.activation(rstd, var, AF.Ln, bias=epsb, scale=1.0)
                nc.scalar.activation(rstd, rstd, AF.Exp, scale=-0.5)
                nc.vector.tensor_tensor(prod, mean, rstd, ALU.mult)
                for i in range(4):
                    qk_sb = qk_tiles[(s, i)]
                    nc.vector.tensor_tensor(
                        qk_sb, qk_sb,
                        st_[:, i, :, 5].to_broadcast([P, 8, DIM]), ALU.mult)
                    nc.vector.tensor_tensor(
                        qk_sb, qk_sb,
                        st_[:, i, :, 6].to_broadcast([P, 8, DIM]),
                        ALU.subtract)

            def unit_tr(s, hl, which):
                """Transpose one head's q or k for supertile s into qt/kt."""
                pr, ro = hl // 2, DIM * (hl % 2)
                dst = (qt, kt)[which]
                tp = ps.tile([DIM, 512], bf16, tag="pj", name="tp_ps")
                for i in range(4):
                    nc.tensor.transpose(
                        tp[:, i * P:(i + 1) * P],
                        qk_tiles[(s, i)][:, 2 * hl + which],
                        ident,
                    )
                nc.vector.tensor_copy(
                    dst[pr][ro:ro + DIM, s * 512:(s + 1) * 512], tp)

            def unit_st(s, pr, j):
                """Scores + exp for head-pair pr, k-tile j, q-supertile s."""
                ls = slice(s * 512, (s + 1) * 512)
                pp = j - 4 * s
                woff = max(0, pp) * P
                stp = ps.tile([P, 2, 512], f32, tag="st", name="st_ps")
                for r01 in range(2):
                    ro = DIM * r01
                    nc.tensor.matmul(
                        stp[:, r01],
                        kt[pr][ro:ro + DIM, j * P:(j + 1) * P],
                        qt[pr][ro:ro + DIM, ls],
                        start=True, stop=True, tile_position=(ro, 0),
                    )
                es = esp.tile([P, 2, 512], bf16, tag="es", bufs=8,
                              name="es_t")
                nc.scalar.activation(es[:, :, woff:], stp[:, :, woff:],
                                     AF.Exp, scale=1.0 / DIM)
                if pp >= 0:
                    blk = slice(pp * P, (pp + 1) * P)
                    for r01 in range(2):
                        nc.gpsimd.tensor_tensor(
                            es[:, r01, blk], es[:, r01, blk], up01, ALU.mult)
                return es

            def unit_av(pr, j, es, av_ps, njs, s):
                woff = max(0, j - 4 * s) * P
                for r01 in range(2):
                    hl = 2 * pr + r01
                    nc.tensor.matmul(
                        av_ps[:, r01, woff:],
                        v_sb[:, j, hl],
                        es[:, r01, woff:],
                        start=(j == 0), stop=(j == njs - 1),
                    )

            def unit_out(t, tail=False):
                """Out-projection for l-tile t (all 4 heads, at supertile)."""
                s = t // 4
                o = outp.tile([P, D], f32, tag="o", name="o_t")
                for half in range(2):
                    op_ps = ps.tile([P, 512], f32, tag="pj", name="op_ps")
                    for c in range(2):
                        nc.tensor.matmul(
                            op_ps,
                            at[c][:, t * P:(t + 1) * P],
                            wout[:, c, half * 512:(half + 1) * 512],
                            start=(c == 0), stop=(c == 1),
                        )
                    # zipped units evacuate on DVE (ACT is exp-saturated
                    # mid-loop); tail units use the then-idle ACT
                    if tail:
                        nc.scalar.copy(o[:, half * 512:(half + 1) * 512],
                                       op_ps)
                    else:
                        nc.vector.tensor_copy(
                            o[:, half * 512:(half + 1) * 512], op_ps)
                nc.sync.dma_start(OUT[t * P:(t + 1) * P, :], o)

            # ---- emission with background-unit zipper ----

            def a_units(s):
                u = []
                for i in range(4):
                    u.append(lambda t=4 * s + i: unit_proj(t))
                u.append(lambda s=s: unit_ln_finish(s))
                for hl in range(NHL):
                    for which in range(2):
                        u.append(lambda s=s, hl=hl, w=which: unit_tr(s, hl, w))
                return u

            fin_state = {}  # s -> (den_t, av_sbs)

            def unit_recip(s):
                # pr-group pr's two denominator rows live at partition 32*pr,
                # free slot r01. Partitions 1..31 are never written or read —
                # the approx-reciprocal runs on rows 0..32 and only rows 0/32
                # are consumed.
                den_t, _ = fin_state[s]
                denf = esp.tile([P, 2, 512], f32, tag="denf", bufs=2,
                                name="denf_t")
                denr = esp.tile([P, 2, 512], f32r, tag="denr", bufs=2,
                                name="denr_t")
                nc.vector.reciprocal_approx_fast(denf[0:33], den_t[0:33])
                with nc.allow_low_precision(
                        reason="fp32r rounding of softmax recip"):
                    nc.vector.tensor_copy(denr[0:33], denf[0:33])
                fin_state[s] = (denr, fin_state[s][1])

            def unit_at(s, hl):
                denr, av_sbs = fin_state[s]
                pr, r01 = hl // 2, hl % 2
                ro = DIM * r01
                bc = ps.tile([DIM, 512], f32, tag="pj", name="bc_ps")
                bp = 32 * pr
                nc.tensor.matmul(bc, ones_all[bp:bp + 1, :],
                                 denr[bp:bp + 1, r01, :],
                                 start=True, stop=True,
                                 tile_position=(bp, 0))
                nc.vector.tensor_tensor(
                    at[pr][ro:ro + DIM, s * 512:(s + 1) * 512],
                    av_sbs[pr][:, r01], bc, ALU.mult)

            def fin_units(s):
                u = [lambda s=s: unit_recip(s)]
                for hl in range(NHL):
                    u.append(lambda s=s, hl=hl: unit_at(s, hl))
                return u

            # supertile 0 head: emit projections/LN and the pr0 (heads 0/1)
            # transposes inline; defer the pr1 transposes into BC(0)'s
            # background queue so pr0's j-loop starts ~4us earlier.
            a0 = a_units(0)
            for u in a0[:9]:
                u()
            a0_tail = a0[9:]

            for s in range(SUP):
                # bg_a (next supertile's projection/LN/transpose chain, plus
                # the finalize of the previous one) is front-loaded into the
                # first ~55% of j-slots so its multi-engine LN latency clears
                # well before BC(s+1) needs qt/kt; bg_c (out-projections,
                # lagging two supertiles) spreads across the whole loop.
                bg_a = []
                if s == 0:
                    bg_a.extend(a0_tail)
                if s + 2 < SUP:
                    bg_a.append(lambda ss=s + 2: dma_xt(ss))
                if s >= 1:
                    bg_a.extend(fin_units(s - 1))
                if s + 1 < SUP:
                    bg_a.extend(a_units(s + 1))
                bg_c = []
                for so in ([0, 1, 2] if s == 3 else []):
                    for t in range(4 * so, 4 * so + 4):
                        bg_c.append(lambda t=t: unit_out(t))

                njs = 4 * s + 4
                n_slots = 2 * njs
                pace_a = len(bg_a) / max(1.0, 0.8 * n_slots)
                pace_c = len(bg_c) / (1.2 * n_slots)
                acc_a = 0.0
                acc_c = 0.0

                den_t = esp.tile([P, 2, 512], f32, tag="den", bufs=2,
                                 name="den_t")
                av_sbs = {}
                for pr in range(2):
                    if pr == 1:  # cover the av-pool turnaround
                        for q in (bg_a, bg_c):
                            if q:
                                q.pop(0)()
                                break
                    av_ps = ps.tile([DIM + 1, 2, 512], f32, tag="av", bufs=1,
                                    name="av_ps")
                    # av trails st by 4 iterations so exp+mask latency is
                    # fully hidden behind four st/bg rounds of PE work
                    pend = []
                    for j in range(njs):
                        es = unit_st(s, pr, j)
                        pend.append((j, es))
                        if len(pend) > 4:
                            jj, ee = pend.pop(0)
                            unit_av(pr, jj, ee, av_ps, njs, s)
                        acc_a += pace_a
                        while acc_a >= 1.0 and bg_a:
                            bg_a.pop(0)()
                            acc_a -= 1.0
                        acc_c += pace_c
                        while acc_c >= 1.0 and bg_c:
                            bg_c.pop(0)()
                            acc_c -= 1.0
                    for jj, ee in pend:
                        unit_av(pr, jj, ee, av_ps, njs, s)
                    # evacuate numerators (bf16) and denominators
                    avs = esp.tile([DIM, 2, 512], bf16, tag="avsb",
                                   bufs=4, name="avs_t")
                    nc.vector.tensor_copy(avs, av_ps[:DIM])
                    nc.vector.tensor_copy(den_t[32 * pr:32 * pr + 1, :, :],
                                          av_ps[DIM:DIM + 1, :, :])
                    av_sbs[pr] = avs
                fin_state[s] = (den_t, av_sbs)
                while bg_a:
                    bg_a.pop(0)()
                while bg_c:
                    bg_c.pop(0)()

            for u in fin_units(SUP - 1):
                u()
            for t in range(4 * 3, 4 * 4):
                unit_out(t, tail=True)

        outp.release()
        esp.release()
        work.release()
        big.release()
        const.release()

    nc.finalize()
    return nc


def _get_nc():
    if "nc" not in _CACHE:
        _CACHE["nc"] = _build_nc()
    return _CACHE["nc"]


def kernel(**inputs):
    import ml_dtypes

    bf = ml_dtypes.bfloat16
    x = np.asarray(inputs["inputs"], dtype=np.float32)
    w_qk = np.asarray(inputs["W_qk"], dtype=np.float32)
    w_v = np.asarray(inputs["W_v"], dtype=np.float32)
    w_out = np.asarray(inputs["W_out"], dtype=np.float32) / 32.0

    # host-side transpose + supertile split: xts[b] is [SUP, D, 512] bf16
    xts = [
        np.ascontiguousarray(
            x[b].T.reshape(D, SUP, 512).transpose(1, 0, 2)).astype(bf)
        for b in range(B)
    ]
    nc = _get_nc()
    in_maps = []
    for c in range(N_CORES):
        b, g = divmod(c, 4)
        in_maps.append({
            "xt": xts[b],
            "w_qk": np.ascontiguousarray(
                w_qk[:, 512 * g:512 * (g + 1)]).astype(bf),
            "w_v": np.ascontiguousarray(
                w_v[:, 256 * g:256 * (g + 1)]).astype(bf),
            "w_out": np.ascontiguousarray(
                w_out[256 * g:256 * (g + 1), :]).astype(bf),
        })

    from concourse.bass_utils import run_bass_kernel_spmd

    trace = bool(os.environ.get("KERNEL_TRACE"))
    if trace:
        try:
            from antenv.axon_hooks import get_axon_ntff_profile_hook  # noqa: F401
        except Exception:
            trace = False
    res = run_bass_kernel_spmd(nc, in_maps, core_ids=list(range(N_CORES)),
                               trace=trace)
    _CACHE["last_results"] = res
    outs = [m["out"] for m in res.results]
    out = np.stack([
        outs[0] + outs[1] + outs[2] + outs[3],
        outs[4] + outs[5] + outs[6] + outs[7],
    ]).astype(np.float32)
    return out


# revision 39
# speedup vs baseline: 1.0041x; 1.0041x over previous
"""Trainium2 Bass kernel for nn_Causal_Attention_13082470383895.

Full (unsharded) inputs in, full output out. Internally shards batch*heads
across 8 NeuronCores: core c owns batch c//4 and the 4 heads [4*(c%4), 4*(c%4)+4).
Each core computes its heads' q/k/v projections (column-sharded weights),
QK-layernorm, causal unnormalized-exp attention, and its partial contribution
to the output projection (row-sharded W_out). Host sums the 4 partials per batch.

Perf notes vs the first working version (414us -> 232us):
- x is transposed on the host, so the kernel DMAs x^T directly and skips the
  128 PE transposes + their PSUM evacuation copies per core.
- everything on the PE runs in bf16 (host-cast); PSUM accumulation stays f32.
- layernorm runs as: bf16 stats (tensor_reduce of qk and qk^2), batched
  Ln/Exp rstd on ScalarE, then two broadcast tensor_tensors per tile on DVE
  (the original per-group GpSimd tensor_scalar path measured ~1.25us/op).
- exp for a head-pair is one ACTIVATE over a 2-bank PSUM tile [128,2,512]
  (halves the per-instruction 352-cycle fixed cost); exp is the dominant
  ScalarE load (~82us) and paces the attention inner loop.
- softmax denominators for a head-pair land on one 32-aligned partition
  (free slot = r01); one reciprocal_approx_fast + f32r cast per supertile,
  then a K=1 PE matmul broadcasts 1/den across 64 partitions per head.
- the av matmul trails its scores matmul by 4 iterations so the
  st -> exp -> mask -> av cross-engine latency is hidden.
- emission zippers projection(s+1), finalize(s-1) and out-projection units
  into attention(s)'s j-loop (front-loaded A-units, out-projections lag into
  supertile 3's long thin loop) to keep the PE dense enough that the HAM
  clock-gate mostly stays at 2.4 GHz.
"""

import os
import sys

import numpy as np

sys.path.insert(0, "/opt/trn_rl_repo")

B = 2
L = 2048
D = 1024
HEADS = 16
DIM = 64
LN_EPS = 1e-6
P = 128
LT = L // P          # 16 l-tiles
DT = D // P          # 8 contraction tiles
NHL = 4              # heads per core
SUP = 4              # 512-wide l supertiles
N_CORES = 8
EPS_RAW = float(D * LN_EPS)  # LN eps folded for raw (unscaled) qk

_CACHE = {}


def _make_bacc_cls():
    import bass_rust
    import concourse.mybir as mybir
    from concourse import bacc
    from concourse.hw_specs import get_activation_tables

    class KernelBacc(bacc.Bacc):
        """Bacc whose ACT-table selector never picks the `natural_log` set
        for Ln: hiding `ln` there makes the greedy selector choose
        `natural_log_exp_and_others` (which also holds exp/copy), so the
        kernel needs a single table load instead of thrashing
        exp_and_others <-> natural_log on every layernorm."""

        def insert_act_table_loads(self):
            has_activation = any(
                isinstance(i, mybir.InstActivation)
                for b in self.main_func.blocks
                for i in b.instructions
            )
            if not has_activation:
                return
            ln = mybir.ActivationFunctionType.Ln
            tables = []
            for name, funcs in get_activation_tables(self.m.arch).items():
                if name == "natural_log":
                    funcs = funcs - {ln}
                tables.append((name, funcs))
            bass_rust.insert_act_table_loads(self, tables)

    return KernelBacc


def _build_nc():
    import concourse.bass as bass  # noqa: F401
    import concourse.mybir as mybir
    import concourse.tile as tile
    from concourse.masks import make_identity, make_upper_triangular

    f32 = mybir.dt.float32
    f32r = mybir.dt.float32r
    bf16 = mybir.dt.bfloat16
    AF = mybir.ActivationFunctionType
    ALU = mybir.AluOpType

    nc = _make_bacc_cls()("TRN2", target_bir_lowering=False, debug=False)

    XT = nc.dram_tensor("xt", [SUP, D, 512], bf16, kind="ExternalInput").ap()
    WQK = nc.dram_tensor("w_qk", [D, 512], bf16, kind="ExternalInput").ap()
    WV = nc.dram_tensor("w_v", [D, 256], bf16, kind="ExternalInput").ap()
    WOUT = nc.dram_tensor("w_out", [256, D], bf16, kind="ExternalInput").ap()
    OUT = nc.dram_tensor("out", [L, D], f32, kind="ExternalOutput").ap()

    with tile.TileContext(nc) as tc:
        const = tc.alloc_tile_pool(name="const", bufs=1)
        big = tc.alloc_tile_pool(name="big", bufs=1)
        work = tc.alloc_tile_pool(name="work", bufs=2)
        esp = tc.alloc_tile_pool(name="esp", bufs=4)
        outp = tc.alloc_tile_pool(name="outp", bufs=2)

        # Input DMAs are dispatched first so the transfers overlap the
        # const setup + engine-boot preamble below.
        xt = [big.tile([P, DT, 512], bf16, name=f"xt{s}") for s in range(SUP)]
        wqk = big.tile([P, DT, 512], bf16)
        wv = big.tile([P, DT, 256], bf16)
        wout = big.tile([P, 2, D], bf16)

        def dma_xt(s):
            nc.sync.dma_start(xt[s], XT[s].rearrange("(o p) l -> p o l", p=P))

        # chunk the first x^T / wqk transfers so the first projection matmul
        # is gated on ~1MB, not the full 3MB prologue
        nc.sync.dma_start(xt[0][:, :2],
                          XT[0][:256].rearrange("(o p) l -> p o l", p=P))
        nc.sync.dma_start(wqk[:, :2],
                          WQK[:256].rearrange("(o p) n -> p o n", p=P))
        nc.sync.dma_start(xt[0][:, 2:],
                          XT[0][256:].rearrange("(o p) l -> p o l", p=P))
        nc.sync.dma_start(wqk[:, 2:],
                          WQK[256:].rearrange("(o p) n -> p o n", p=P))
        nc.sync.dma_start(wv, WV.rearrange("(o p) n -> p o n", p=P))
        dma_xt(1)
        nc.sync.dma_start(wout, WOUT.rearrange("(c p) n -> p c n", p=P))

        ident = const.tile([P, P], bf16)
        make_identity(nc, ident)
        # 0/1 upper-triangular (incl diagonal) for post-exp causal masking of
        # the diagonal 128x128 block: es layout is S^T (k on partitions), so
        # valid = (q >= k) = upper triangle.
        up01 = const.tile([P, P], bf16)
        make_upper_triangular(nc, up01, val=1.0, diag=True)
        epsb = const.tile([P, 1], f32)
        nc.vector.memset(epsb, EPS_RAW)
        ones_bf = const.tile([P, 1], bf16)
        nc.vector.memset(ones_bf, 1.0)
        # stationary for the K=1 denominator-broadcast matmul; sliced at the
        # moving operand's base partition (they must match). f32r tiles can't
        # be memset directly — cast from f32 via DVE.
        ones_f32 = const.tile([P, 1], f32)
        nc.vector.memset(ones_f32, 1.0)
        ones_all = const.tile([P, DIM], f32r)
        nc.vector.tensor_copy(ones_all, ones_f32[:, 0:1].to_broadcast([P, DIM]))

        # Collapse const-setup waits behind one barrier (wait-slot limits).
        tc.strict_bb_all_engine_barrier()

        # persistent intermediates. qt/kt/at pair 2 heads on the partition
        # axis: head 2i in rows 0:64, head 2i+1 in rows 64:128.
        # v is stored augmented per head: [v_h | 1] (65 cols) so one AV
        # matmul yields both the numerator (rows 0:64) and the softmax
        # denominator (row 64).
        v_sb = big.tile([P, LT, NHL, DIM + 1], bf16)
        qt = [big.tile([P, L], bf16, name=f"qt{i}") for i in range(2)]
        kt = [big.tile([P, L], bf16, name=f"kt{i}") for i in range(2)]
        at = [big.tile([P, L], bf16, name=f"at{i}") for i in range(2)]
        nc.vector.tensor_copy(
            v_sb[:, :, :, DIM],
            ones_bf[:, 0:1].to_broadcast([P, LT, NHL]),
        )

        qk_tiles = {}   # (s, i) -> qk_sb tile
        stat_tiles = {}  # s -> per-supertile stat tile

        with tc.tile_pool(name="ps", bufs=2, space="PSUM") as ps:
            # PSUM budget (8 banks): pj 2 + st 2x[128,2,512] (4 banks) +
            # av 1x[65,2,512] (2 banks). tp/bc/op share the pj tag.

            def unit_proj(t):
                """Projection + LN stats for l-tile t."""
                s, i = t // 4, t % 4
                xts = xt[s]
                qk_ps = ps.tile([P, 512], f32, tag="pj", name="qk_ps")
                for d in range(DT):
                    nc.tensor.matmul(
                        qk_ps, xts[:, d, i * P:(i + 1) * P], wqk[:, d],
                        start=(d == 0), stop=(d == DT - 1),
                    )
                v_ps = ps.tile([P, 256], f32, tag="pj", name="v_ps")
                for d in range(DT):
                    nc.tensor.matmul(
                        v_ps, xts[:, d, i * P:(i + 1) * P], wv[:, d],
                        start=(d == 0), stop=(d == DT - 1),
                    )
                qk_sb = work.tile([P, 8, DIM], bf16, tag="qk", bufs=6,
                                  name="qk_sb")
                nc.scalar.copy(
                    qk_sb, qk_ps.rearrange("p (g d) -> p g d", g=8))
                qk_tiles[(s, i)] = qk_sb
                # v scaled by 1/sqrt(D)=1/32 here; the out-proj 1/32 is
                # folded into wout on the host.
                nc.scalar.mul(
                    v_sb[:, t, :, :DIM],
                    v_ps.rearrange("p (h d) -> p h d", h=NHL), 1.0 / 32.0)
                # LN stats over each 64-group (raw qk: eps folded as D*eps)
                if s not in stat_tiles:
                    stat_tiles[s] = work.tile([P, 4, 8, 8], bf16, tag="stat",
                                              bufs=2, name="stat_t")
                st_ = stat_tiles[s]
                sq = work.tile([P, 8, DIM], bf16, tag="sq", bufs=2,
                               name="sq_t")
                nc.vector.tensor_tensor(sq, qk_sb, qk_sb, ALU.mult)
                with nc.allow_low_precision(
                        reason="bf16 LN stats; var >> mean^2 here"):
                    nc.vector.tensor_reduce(
                        st_[:, i, :, 0], qk_sb, axis=mybir.AxisListType.X,
                        op=ALU.add)
                    nc.vector.tensor_reduce(
                        st_[:, i, :, 1], sq, axis=mybir.AxisListType.X,
                        op=ALU.add)

            def unit_ln_finish(s):
                """Batched rstd for all 4 l-tiles of supertile s, then apply."""
                st_ = stat_tiles.pop(s)
                sums = st_[:, :, :, 0]
                sumsq = st_[:, :, :, 1]
                mean = st_[:, :, :, 2]
                mn2 = st_[:, :, :, 3]
                var = st_[:, :, :, 4]
                rstd = st_[:, :, :, 5]
                prod = st_[:, :, :, 6]
                nc.vector.tensor_scalar_mul(mean, sums, 1.0 / DIM)
                nc.vector.tensor_tensor(mn2, mean, mean, ALU.mult)
                nc.vector.tensor_scalar_mul(var, sumsq, 1.0 / DIM)
                nc.vector.tensor_tensor(var, var, mn2, ALU.subtract)
                nc.scalar.activation(rstd, var, AF.Ln, bias=epsb, scale=1.0)
                nc.scalar.activation(rstd, rstd, AF.Exp, scale=-0.5)
                nc.vector.tensor_tensor(prod, mean, rstd, ALU.mult)
                for i in range(4):
                    qk_sb = qk_tiles[(s, i)]
                    nc.vector.tensor_tensor(
                        qk_sb, qk_sb,
                        st_[:, i, :, 5].to_broadcast([P, 8, DIM]), ALU.mult)
                    nc.vector.tensor_tensor(
                        qk_sb, qk_sb,
                        st_[:, i, :, 6].to_broadcast([P, 8, DIM]),
                        ALU.subtract)

            def unit_tr(s, hl, which):
                """Transpose one head's q or k for supertile s into qt/kt."""
                pr, ro = hl // 2, DIM * (hl % 2)
                dst = (qt, kt)[which]
                tp = ps.tile([DIM, 512], bf16, tag="pj", name="tp_ps")
                for i in range(4):
                    nc.tensor.transpose(
                        tp[:, i * P:(i + 1) * P],
                        qk_tiles[(s, i)][:, 2 * hl + which],
                        ident,
                    )
                nc.vector.tensor_copy(
                    dst[pr][ro:ro + DIM, s * 512:(s + 1) * 512], tp)

            def unit_st(s, pr, j):
                """Scores + exp for head-pair pr, k-tile j, q-supertile s."""
                ls = slice(s * 512, (s + 1) * 512)
                pp = j - 4 * s
                woff = max(0, pp) * P
                stp = ps.tile([P, 2, 512], f32, tag="st", name="st_ps")
                for r01 in range(2):
                    ro = DIM * r01
                    nc.tensor.matmul(
                        stp[:, r01],
                        kt[pr][ro:ro + DIM, j * P:(j + 1) * P],
                        qt[pr][ro:ro + DIM, ls],
                        start=True, stop=True, tile_position=(ro, 0),
                    )
                es = esp.tile([P, 2, 512], bf16, tag="es", bufs=8,
                              name="es_t")
                nc.scalar.activation(es[:, :, woff:], stp[:, :, woff:],
                                     AF.Exp, scale=1.0 / DIM)
                if pp >= 0:
                    blk = slice(pp * P, (pp + 1) * P)
                    for r01 in range(2):
                        nc.gpsimd.tensor_tensor(
                            es[:, r01, blk], es[:, r01, blk], up01, ALU.mult)
                return es

            def unit_av(pr, j, es, av_ps, njs, s):
                woff = max(0, j - 4 * s) * P
                for r01 in range(2):
                    hl = 2 * pr + r01
                    nc.tensor.matmul(
                        av_ps[:, r01, woff:],
                        v_sb[:, j, hl],
                        es[:, r01, woff:],
                        start=(j == 0), stop=(j == njs - 1),
                    )

            def unit_out(t):
                """Out-projection for l-tile t (all 4 heads, at supertile)."""
                s = t // 4
                o = outp.tile([P, D], f32, tag="o", name="o_t")
                for half in range(2):
                    op_ps = ps.tile([P, 512], f32, tag="pj", name="op_ps")
                    for c in range(2):
                        nc.tensor.matmul(
                            op_ps,
                            at[c][:, t * P:(t + 1) * P],
                            wout[:, c, half * 512:(half + 1) * 512],
                            start=(c == 0), stop=(c == 1),
                        )
                    nc.vector.tensor_copy(o[:, half * 512:(half + 1) * 512],
                                          op_ps)
                nc.sync.dma_start(OUT[t * P:(t + 1) * P, :], o)

            # ---- emission with background-unit zipper ----

            def a_units(s):
                u = []
                for i in range(4):
                    u.append(lambda t=4 * s + i: unit_proj(t))
                u.append(lambda s=s: unit_ln_finish(s))
                for hl in range(NHL):
                    for which in range(2):
                        u.append(lambda s=s, hl=hl, w=which: unit_tr(s, hl, w))
                return u

            fin_state = {}  # s -> (den_t, av_sbs)

            def unit_recip(s):
                # pr-group pr's two denominator rows live at partition 32*pr,
                # free slot r01. Partitions 1..31 are never written or read —
                # the approx-reciprocal runs on rows 0..32 and only rows 0/32
                # are consumed.
                den_t, _ = fin_state[s]
                denf = esp.tile([P, 2, 512], f32, tag="denf", bufs=2,
                                name="denf_t")
                denr = esp.tile([P, 2, 512], f32r, tag="denr", bufs=2,
                                name="denr_t")
                nc.vector.reciprocal_approx_fast(denf[0:33], den_t[0:33])
                with nc.allow_low_precision(
                        reason="fp32r rounding of softmax recip"):
                    nc.vector.tensor_copy(denr[0:33], denf[0:33])
                fin_state[s] = (denr, fin_state[s][1])

            def unit_at(s, hl):
                denr, av_sbs = fin_state[s]
                pr, r01 = hl // 2, hl % 2
                ro = DIM * r01
                bc = ps.tile([DIM, 512], f32, tag="pj", name="bc_ps")
                bp = 32 * pr
                nc.tensor.matmul(bc, ones_all[bp:bp + 1, :],
                                 denr[bp:bp + 1, r01, :],
                                 start=True, stop=True,
                                 tile_position=(bp, 0))
                nc.vector.tensor_tensor(
                    at[pr][ro:ro + DIM, s * 512:(s + 1) * 512],
                    av_sbs[pr][:, r01], bc, ALU.mult)

            def fin_units(s):
                u = [lambda s=s: unit_recip(s)]
                for hl in range(NHL):
                    u.append(lambda s=s, hl=hl: unit_at(s, hl))
                return u

            # supertile 0 head: emit projections/LN and the pr0 (heads 0/1)
            # transposes inline; defer the pr1 transposes into BC(0)'s
            # background queue so pr0's j-loop starts ~4us earlier.
            a0 = a_units(0)
            for u in a0[:9]:
                u()
            a0_tail = a0[9:]

            for s in range(SUP):
                # bg_a (next supertile's projection/LN/transpose chain, plus
                # the finalize of the previous one) is front-loaded into the
                # first ~55% of j-slots so its multi-engine LN latency clears
                # well before BC(s+1) needs qt/kt; bg_c (out-projections,
                # lagging two supertiles) spreads across the whole loop.
                bg_a = []
                if s == 0:
                    bg_a.extend(a0_tail)
                if s + 2 < SUP:
                    bg_a.append(lambda ss=s + 2: dma_xt(ss))
                if s >= 1:
                    bg_a.extend(fin_units(s - 1))
                if s + 1 < SUP:
                    bg_a.extend(a_units(s + 1))
                bg_c = []
                for so in ([0, 1, 2] if s == 3 else []):
                    for t in range(4 * so, 4 * so + 4):
                        bg_c.append(lambda t=t: unit_out(t))

                njs = 4 * s + 4
                n_slots = 2 * njs
                pace_a = len(bg_a) / max(1.0, 0.8 * n_slots)
                pace_c = len(bg_c) / n_slots
                acc_a = 0.0
                acc_c = 0.0

                den_t = esp.tile([P, 2, 512], f32, tag="den", bufs=2,
                                 name="den_t")
                av_sbs = {}
                for pr in range(2):
                    if pr == 1:  # cover the av-pool turnaround
                        for q in (bg_a, bg_c):
                            if q:
                                q.pop(0)()
                                break
                    av_ps = ps.tile([DIM + 1, 2, 512], f32, tag="av", bufs=1,
                                    name="av_ps")
                    # av trails st by 4 iterations so exp+mask latency is
                    # fully hidden behind four st/bg rounds of PE work
                    pend = []
                    for j in range(njs):
                        es = unit_st(s, pr, j)
                        pend.append((j, es))
                        if len(pend) > 4:
                            jj, ee = pend.pop(0)
                            unit_av(pr, jj, ee, av_ps, njs, s)
                        acc_a += pace_a
                        while acc_a >= 1.0 and bg_a:
                            bg_a.pop(0)()
                            acc_a -= 1.0
                        acc_c += pace_c
                        while acc_c >= 1.0 and bg_c:
                            bg_c.pop(0)()
                            acc_c -= 1.0
                    for jj, ee in pend:
                        unit_av(pr, jj, ee, av_ps, njs, s)
                    # evacuate numerators (bf16) and denominators
                    avs = esp.tile([DIM, 2, 512], bf16, tag="avsb",
                                   bufs=4, name="avs_t")
                    nc.vector.tensor_copy(avs, av_ps[:DIM])
                    nc.vector.tensor_copy(den_t[32 * pr:32 * pr + 1, :, :],
                                          av_ps[DIM:DIM + 1, :, :])
                    av_sbs[pr] = avs
                fin_state[s] = (den_t, av_sbs)
                while bg_a:
                    bg_a.pop(0)()
                while bg_c:
                    bg_c.pop(0)()

            for u in fin_units(SUP - 1):
                u()
            for t in range(4 * 3, 4 * 4):
                unit_out(t)

        outp.release()
        esp.release()
        work.release()
        big.release()
        const.release()

    nc.finalize()
    return nc


def _get_nc():
    if "nc" not in _CACHE:
        _CACHE["nc"] = _build_nc()
    return _CACHE["nc"]


def kernel(**inputs):
    import ml_dtypes

    bf = ml_dtypes.bfloat16
    x = np.asarray(inputs["inputs"], dtype=np.float32)
    w_qk = np.asarray(inputs["W_qk"], dtype=np.float32)
    w_v = np.asarray(inputs["W_v"], dtype=np.float32)
    w_out = np.asarray(inputs["W_out"], dtype=np.float32) / 32.0

    # host-side transpose + supertile split: xts[b] is [SUP, D, 512] bf16
    xts = [
        np.ascontiguousarray(
            x[b].T.reshape(D, SUP, 512).transpose(1, 0, 2)).astype(bf)
        for b in range(B)
    ]
    nc = _get_nc()
    in_maps = []
    for c in range(N_CORES):
        b, g = divmod(c, 4)
        in_maps.append({
            "xt": xts[b],
            "w_qk": np.ascontiguousarray(
                w_qk[:, 512 * g:512 * (g + 1)]).astype(bf),
            "w_v": np.ascontiguousarray(
                w_v[:, 256 * g:256 * (g + 1)]).astype(bf),
            "w_out": np.ascontiguousarray(
                w_out[256 * g:256 * (g + 1), :]).astype(bf),
        })

    from concourse.bass_utils import run_bass_kernel_spmd

    trace = bool(os.environ.get("KERNEL_TRACE"))
    if trace:
        try:
            from antenv.axon_hooks import get_axon_ntff_profile_hook  # noqa: F401
        except Exception:
            trace = False
    res = run_bass_kernel_spmd(nc, in_maps, core_ids=list(range(N_CORES)),
                               trace=trace)
    _CACHE["last_results"] = res
    outs = [m["out"] for m in res.results]
    out = np.stack([
        outs[0] + outs[1] + outs[2] + outs[3],
        outs[4] + outs[5] + outs[6] + outs[7],
    ]).astype(np.float32)
    return out


# revision 41
# speedup vs baseline: 1.0102x; 1.0061x over previous
"""Trainium2 Bass kernel for nn_Causal_Attention_13082470383895.

Full (unsharded) inputs in, full output out. Internally shards batch*heads
across 8 NeuronCores: core c owns batch c//4 and the 4 heads [4*(c%4), 4*(c%4)+4).
Each core computes its heads' q/k/v projections (column-sharded weights),
QK-layernorm, causal unnormalized-exp attention, and its partial contribution
to the output projection (row-sharded W_out). Host sums the 4 partials per batch.

Perf notes vs the first working version (414us -> 232us):
- x is transposed on the host, so the kernel DMAs x^T directly and skips the
  128 PE transposes + their PSUM evacuation copies per core.
- everything on the PE runs in bf16 (host-cast); PSUM accumulation stays f32.
- layernorm runs as: bf16 stats (tensor_reduce of qk and qk^2), batched
  Ln/Exp rstd on ScalarE, then two broadcast tensor_tensors per tile on DVE
  (the original per-group GpSimd tensor_scalar path measured ~1.25us/op).
- exp for a head-pair is one ACTIVATE over a 2-bank PSUM tile [128,2,512]
  (halves the per-instruction 352-cycle fixed cost); exp is the dominant
  ScalarE load (~82us) and paces the attention inner loop.
- softmax denominators for a head-pair land on one 32-aligned partition
  (free slot = r01); one reciprocal_approx_fast + f32r cast per supertile,
  then a K=1 PE matmul broadcasts 1/den across 64 partitions per head.
- the av matmul trails its scores matmul by 4 iterations so the
  st -> exp -> mask -> av cross-engine latency is hidden.
- emission zippers projection(s+1), finalize(s-1) and out-projection units
  into attention(s)'s j-loop (front-loaded A-units, out-projections lag into
  supertile 3's long thin loop) to keep the PE dense enough that the HAM
  clock-gate mostly stays at 2.4 GHz.
"""

import os
import sys

import numpy as np

sys.path.insert(0, "/opt/trn_rl_repo")

B = 2
L = 2048
D = 1024
HEADS = 16
DIM = 64
LN_EPS = 1e-6
P = 128
LT = L // P          # 16 l-tiles
DT = D // P          # 8 contraction tiles
NHL = 4              # heads per core
SUP = 4              # 512-wide l supertiles
N_CORES = 8
EPS_RAW = float(D * LN_EPS)  # LN eps folded for raw (unscaled) qk

_CACHE = {}


def _make_bacc_cls():
    import bass_rust
    import concourse.mybir as mybir
    from concourse import bacc
    from concourse.hw_specs import get_activation_tables

    class KernelBacc(bacc.Bacc):
        """Bacc whose ACT-table selector never picks the `natural_log` set
        for Ln: hiding `ln` there makes the greedy selector choose
        `natural_log_exp_and_others` (which also holds exp/copy), so the
        kernel needs a single table load instead of thrashing
        exp_and_others <-> natural_log on every layernorm."""

        def insert_act_table_loads(self):
            has_activation = any(
                isinstance(i, mybir.InstActivation)
                for b in self.main_func.blocks
                for i in b.instructions
            )
            if not has_activation:
                return
            ln = mybir.ActivationFunctionType.Ln
            tables = []
            for name, funcs in get_activation_tables(self.m.arch).items():
                if name == "natural_log":
                    funcs = funcs - {ln}
                tables.append((name, funcs))
            bass_rust.insert_act_table_loads(self, tables)

    return KernelBacc


def _build_nc():
    import concourse.bass as bass  # noqa: F401
    import concourse.mybir as mybir
    import concourse.tile as tile
    from concourse.masks import make_identity, make_upper_triangular

    f32 = mybir.dt.float32
    f32r = mybir.dt.float32r
    bf16 = mybir.dt.bfloat16
    AF = mybir.ActivationFunctionType
    ALU = mybir.AluOpType

    nc = _make_bacc_cls()("TRN2", target_bir_lowering=False, debug=False)

    XT = nc.dram_tensor("xt", [SUP, D, 512], bf16, kind="ExternalInput").ap()
    WQK = nc.dram_tensor("w_qk", [D, 512], bf16, kind="ExternalInput").ap()
    WV = nc.dram_tensor("w_v", [D, 256], bf16, kind="ExternalInput").ap()
    WOUT = nc.dram_tensor("w_out", [256, D], bf16, kind="ExternalInput").ap()
    OUT = nc.dram_tensor("out", [L, D], f32, kind="ExternalOutput").ap()

    with tile.TileContext(nc) as tc:
        const = tc.alloc_tile_pool(name="const", bufs=1)
        big = tc.alloc_tile_pool(name="big", bufs=1)
        work = tc.alloc_tile_pool(name="work", bufs=2)
        esp = tc.alloc_tile_pool(name="esp", bufs=4)
        outp = tc.alloc_tile_pool(name="outp", bufs=2)

        # Input DMAs are dispatched first so the transfers overlap the
        # const setup + engine-boot preamble below.
        xt = [big.tile([P, DT, 512], bf16, name=f"xt{s}") for s in range(SUP)]
        wqk = big.tile([P, DT, 512], bf16)
        wv = big.tile([P, DT, 256], bf16)
        wout = big.tile([P, 2, D], bf16)

        def dma_xt(s):
            nc.sync.dma_start(xt[s], XT[s].rearrange("(o p) l -> p o l", p=P))

        # chunk the first x^T / wqk transfers so the first projection matmul
        # is gated on ~1MB, not the full 3MB prologue
        nc.sync.dma_start(xt[0][:, :1],
                          XT[0][:128].rearrange("(o p) l -> p o l", p=P))
        nc.sync.dma_start(wqk[:, :1],
                          WQK[:128].rearrange("(o p) n -> p o n", p=P))
        nc.sync.dma_start(xt[0][:, 1:],
                          XT[0][128:].rearrange("(o p) l -> p o l", p=P))
        nc.sync.dma_start(wqk[:, 1:],
                          WQK[128:].rearrange("(o p) n -> p o n", p=P))
        nc.sync.dma_start(wv, WV.rearrange("(o p) n -> p o n", p=P))
        dma_xt(1)
        nc.sync.dma_start(wout, WOUT.rearrange("(c p) n -> p c n", p=P))

        ident = const.tile([P, P], bf16)
        make_identity(nc, ident)
        # 0/1 upper-triangular (incl diagonal) for post-exp causal masking of
        # the diagonal 128x128 block: es layout is S^T (k on partitions), so
        # valid = (q >= k) = upper triangle.
        up01 = const.tile([P, P], bf16)
        make_upper_triangular(nc, up01, val=1.0, diag=True)
        epsb = const.tile([P, 1], f32)
        nc.vector.memset(epsb, EPS_RAW)
        ones_bf = const.tile([P, 1], bf16)
        nc.vector.memset(ones_bf, 1.0)
        # stationary for the K=1 denominator-broadcast matmul; sliced at the
        # moving operand's base partition (they must match). f32r tiles can't
        # be memset directly — cast from f32 via DVE.
        ones_f32 = const.tile([P, 1], f32)
        nc.vector.memset(ones_f32, 1.0)
        ones_all = const.tile([P, DIM], f32r)
        nc.vector.tensor_copy(ones_all, ones_f32[:, 0:1].to_broadcast([P, DIM]))

        # Collapse const-setup waits behind one barrier (wait-slot limits).
        tc.strict_bb_all_engine_barrier()

        # persistent intermediates. qt/kt/at pair 2 heads on the partition
        # axis: head 2i in rows 0:64, head 2i+1 in rows 64:128.
        # v is stored augmented per head: [v_h | 1] (65 cols) so one AV
        # matmul yields both the numerator (rows 0:64) and the softmax
        # denominator (row 64).
        v_sb = big.tile([P, LT, NHL, DIM + 1], bf16)
        qt = [big.tile([P, L], bf16, name=f"qt{i}") for i in range(2)]
        kt = [big.tile([P, L], bf16, name=f"kt{i}") for i in range(2)]
        at = [big.tile([P, L], bf16, name=f"at{i}") for i in range(2)]
        nc.vector.tensor_copy(
            v_sb[:, :, :, DIM],
            ones_bf[:, 0:1].to_broadcast([P, LT, NHL]),
        )

        qk_tiles = {}   # (s, i) -> qk_sb tile
        stat_tiles = {}  # s -> per-supertile stat tile

        with tc.tile_pool(name="ps", bufs=2, space="PSUM") as ps:
            # PSUM budget (8 banks): pj 2 + st 2x[128,2,512] (4 banks) +
            # av 1x[65,2,512] (2 banks). tp/bc/op share the pj tag.

            def unit_proj(t):
                """Projection + LN stats for l-tile t."""
                s, i = t // 4, t % 4
                xts = xt[s]
                qk_ps = ps.tile([P, 512], f32, tag="pj", name="qk_ps")
                for d in range(DT):
                    nc.tensor.matmul(
                        qk_ps, xts[:, d, i * P:(i + 1) * P], wqk[:, d],
                        start=(d == 0), stop=(d == DT - 1),
                    )
                v_ps = ps.tile([P, 256], f32, tag="pj", name="v_ps")
                for d in range(DT):
                    nc.tensor.matmul(
                        v_ps, xts[:, d, i * P:(i + 1) * P], wv[:, d],
                        start=(d == 0), stop=(d == DT - 1),
                    )
                qk_sb = work.tile([P, 8, DIM], bf16, tag="qk", bufs=6,
                                  name="qk_sb")
                nc.scalar.copy(
                    qk_sb, qk_ps.rearrange("p (g d) -> p g d", g=8))
                qk_tiles[(s, i)] = qk_sb
                # v scaled by 1/sqrt(D)=1/32 here; the out-proj 1/32 is
                # folded into wout on the host.
                nc.scalar.mul(
                    v_sb[:, t, :, :DIM],
                    v_ps.rearrange("p (h d) -> p h d", h=NHL), 1.0 / 32.0)
                # LN stats over each 64-group (raw qk: eps folded as D*eps)
                if s not in stat_tiles:
                    stat_tiles[s] = work.tile([P, 4, 8, 8], bf16, tag="stat",
                                              bufs=2, name="stat_t")
                st_ = stat_tiles[s]
                sq = work.tile([P, 8, DIM], bf16, tag="sq", bufs=2,
                               name="sq_t")
                nc.vector.tensor_tensor(sq, qk_sb, qk_sb, ALU.mult)
                with nc.allow_low_precision(
                        reason="bf16 LN stats; var >> mean^2 here"):
                    nc.vector.tensor_reduce(
                        st_[:, i, :, 0], qk_sb, axis=mybir.AxisListType.X,
                        op=ALU.add)
                    nc.vector.tensor_reduce(
                        st_[:, i, :, 1], sq, axis=mybir.AxisListType.X,
                        op=ALU.add)

            def unit_ln_finish(s):
                """Batched rstd for all 4 l-tiles of supertile s, then apply."""
                st_ = stat_tiles.pop(s)
                sums = st_[:, :, :, 0]
                sumsq = st_[:, :, :, 1]
                mean = st_[:, :, :, 2]
                mn2 = st_[:, :, :, 3]
                var = st_[:, :, :, 4]
                rstd = st_[:, :, :, 5]
                prod = st_[:, :, :, 6]
                nc.vector.tensor_scalar_mul(mean, sums, 1.0 / DIM)
                nc.vector.tensor_tensor(mn2, mean, mean, ALU.mult)
                nc.vector.tensor_scalar_mul(var, sumsq, 1.0 / DIM)
                nc.vector.tensor_tensor(var, var, mn2, ALU.subtract)
                nc.scalar.activation(rstd, var, AF.Ln, bias=epsb, scale=1.0)
                nc.scalar.activation(rstd, rstd, AF.Exp, scale=-0.5)
                nc.vector.tensor_tensor(prod, mean, rstd, ALU.mult)
                for i in range(4):
                    qk_sb = qk_tiles[(s, i)]
                    nc.vector.tensor_tensor(
                        qk_sb, qk_sb,
                        st_[:, i, :, 5].to_broadcast([P, 8, DIM]), ALU.mult)
                    nc.vector.tensor_tensor(
                        qk_sb, qk_sb,
                        st_[:, i, :, 6].to_broadcast([P, 8, DIM]),
                        ALU.subtract)

            def unit_tr(s, hl, which):
                """Transpose one head's q or k for supertile s into qt/kt."""
                pr, ro = hl // 2, DIM * (hl % 2)
                dst = (qt, kt)[which]
                tp = ps.tile([DIM, 512], bf16, tag="pj", name="tp_ps")
                for i in range(4):
                    nc.tensor.transpose(
                        tp[:, i * P:(i + 1) * P],
                        qk_tiles[(s, i)][:, 2 * hl + which],
                        ident,
                    )
                nc.vector.tensor_copy(
                    dst[pr][ro:ro + DIM, s * 512:(s + 1) * 512], tp)

            def unit_st(s, pr, j):
                """Scores + exp for head-pair pr, k-tile j, q-supertile s."""
                ls = slice(s * 512, (s + 1) * 512)
                pp = j - 4 * s
                woff = max(0, pp) * P
                stp = ps.tile([P, 2, 512], f32, tag="st", name="st_ps")
                for r01 in range(2):
                    ro = DIM * r01
                    nc.tensor.matmul(
                        stp[:, r01],
                        kt[pr][ro:ro + DIM, j * P:(j + 1) * P],
                        qt[pr][ro:ro + DIM, ls],
                        start=True, stop=True, tile_position=(ro, 0),
                    )
                es = esp.tile([P, 2, 512], bf16, tag="es", bufs=8,
                              name="es_t")
                nc.scalar.activation(es[:, :, woff:], stp[:, :, woff:],
                                     AF.Exp, scale=1.0 / DIM)
                if pp >= 0:
                    blk = slice(pp * P, (pp + 1) * P)
                    for r01 in range(2):
                        nc.gpsimd.tensor_tensor(
                            es[:, r01, blk], es[:, r01, blk], up01, ALU.mult)
                return es

            def unit_av(pr, j, es, av_ps, njs, s):
                woff = max(0, j - 4 * s) * P
                for r01 in range(2):
                    hl = 2 * pr + r01
                    nc.tensor.matmul(
                        av_ps[:, r01, woff:],
                        v_sb[:, j, hl],
                        es[:, r01, woff:],
                        start=(j == 0), stop=(j == njs - 1),
                    )

            def unit_out(t):
                """Out-projection for l-tile t (all 4 heads, at supertile)."""
                s = t // 4
                o = outp.tile([P, D], f32, tag="o", name="o_t")
                for half in range(2):
                    op_ps = ps.tile([P, 512], f32, tag="pj", name="op_ps")
                    for c in range(2):
                        nc.tensor.matmul(
                            op_ps,
                            at[c][:, t * P:(t + 1) * P],
                            wout[:, c, half * 512:(half + 1) * 512],
                            start=(c == 0), stop=(c == 1),
                        )
                    nc.vector.tensor_copy(o[:, half * 512:(half + 1) * 512],
                                          op_ps)
                nc.sync.dma_start(OUT[t * P:(t + 1) * P, :], o)

            # ---- emission with background-unit zipper ----

            def a_units(s):
                u = []
                for i in range(4):
                    u.append(lambda t=4 * s + i: unit_proj(t))
                u.append(lambda s=s: unit_ln_finish(s))
                for hl in range(NHL):
                    for which in range(2):
                        u.append(lambda s=s, hl=hl, w=which: unit_tr(s, hl, w))
                return u

            fin_state = {}  # s -> (den_t, av_sbs)

            def unit_recip(s):
                # pr-group pr's two denominator rows live at partition 32*pr,
                # free slot r01. Partitions 1..31 are never written or read —
                # the approx-reciprocal runs on rows 0..32 and only rows 0/32
                # are consumed.
                den_t, _ = fin_state[s]
                denf = esp.tile([P, 2, 512], f32, tag="denf", bufs=2,
                                name="denf_t")
                denr = esp.tile([P, 2, 512], f32r, tag="denr", bufs=2,
                                name="denr_t")
                nc.vector.reciprocal_approx_fast(denf[0:33], den_t[0:33])
                with nc.allow_low_precision(
                        reason="fp32r rounding of softmax recip"):
                    nc.vector.tensor_copy(denr[0:33], denf[0:33])
                fin_state[s] = (denr, fin_state[s][1])

            def unit_at(s, hl):
                denr, av_sbs = fin_state[s]
                pr, r01 = hl // 2, hl % 2
                ro = DIM * r01
                bc = ps.tile([DIM, 512], f32, tag="pj", name="bc_ps")
                bp = 32 * pr
                nc.tensor.matmul(bc, ones_all[bp:bp + 1, :],
                                 denr[bp:bp + 1, r01, :],
                                 start=True, stop=True,
                                 tile_position=(bp, 0))
                nc.vector.tensor_tensor(
                    at[pr][ro:ro + DIM, s * 512:(s + 1) * 512],
                    av_sbs[pr][:, r01], bc, ALU.mult)

            def fin_units(s):
                u = [lambda s=s: unit_recip(s)]
                for hl in range(NHL):
                    u.append(lambda s=s, hl=hl: unit_at(s, hl))
                return u

            # supertile 0 head: emit projections/LN and the pr0 (heads 0/1)
            # transposes inline; defer the pr1 transposes into BC(0)'s
            # background queue so pr0's j-loop starts ~4us earlier.
            a0 = a_units(0)
            for u in a0[:9]:
                u()
            a0_tail = a0[9:]

            for s in range(SUP):
                # bg_a (next supertile's projection/LN/transpose chain, plus
                # the finalize of the previous one) is front-loaded into the
                # first ~55% of j-slots so its multi-engine LN latency clears
                # well before BC(s+1) needs qt/kt; bg_c (out-projections,
                # lagging two supertiles) spreads across the whole loop.
                bg_a = []
                if s == 0:
                    bg_a.extend(a0_tail)
                if s + 2 < SUP:
                    bg_a.append(lambda ss=s + 2: dma_xt(ss))
                if s >= 1:
                    bg_a.extend(fin_units(s - 1))
                if s + 1 < SUP:
                    bg_a.extend(a_units(s + 1))
                bg_c = []
                for so in ([0, 1, 2] if s == 3 else []):
                    for t in range(4 * so, 4 * so + 4):
                        bg_c.append(lambda t=t: unit_out(t))

                njs = 4 * s + 4
                n_slots = 2 * njs
                pace_a = len(bg_a) / max(1.0, 0.8 * n_slots)
                pace_c = len(bg_c) / n_slots
                acc_a = 0.0
                acc_c = 0.0

                den_t = esp.tile([P, 2, 512], f32, tag="den", bufs=2,
                                 name="den_t")
                av_sbs = {}
                for pr in range(2):
                    if pr == 1:  # cover the av-pool turnaround
                        for q in (bg_a, bg_c):
                            if q:
                                q.pop(0)()
                                break
                    av_ps = ps.tile([DIM + 1, 2, 512], f32, tag="av", bufs=1,
                                    name="av_ps")
                    # av trails st by 5 iterations so exp+mask latency is
                    # fully hidden behind five st/bg rounds of PE work
                    pend = []
                    for j in range(njs):
                        es = unit_st(s, pr, j)
                        pend.append((j, es))
                        if len(pend) > 5:
                            jj, ee = pend.pop(0)
                            unit_av(pr, jj, ee, av_ps, njs, s)
                        acc_a += pace_a
                        while acc_a >= 1.0 and bg_a:
                            bg_a.pop(0)()
                            acc_a -= 1.0
                        acc_c += pace_c
                        while acc_c >= 1.0 and bg_c:
                            bg_c.pop(0)()
                            acc_c -= 1.0
                    for jj, ee in pend:
                        unit_av(pr, jj, ee, av_ps, njs, s)
                    # evacuate numerators (bf16) and denominators
                    avs = esp.tile([DIM, 2, 512], bf16, tag="avsb",
                                   bufs=4, name="avs_t")
                    nc.vector.tensor_copy(avs, av_ps[:DIM])
                    nc.vector.tensor_copy(den_t[32 * pr:32 * pr + 1, :, :],
                                          av_ps[DIM:DIM + 1, :, :])
                    av_sbs[pr] = avs
                fin_state[s] = (den_t, av_sbs)
                while bg_a:
                    bg_a.pop(0)()
                while bg_c:
                    bg_c.pop(0)()

            for u in fin_units(SUP - 1):
                u()
            for t in range(4 * 3, 4 * 4):
                unit_out(t)

        outp.release()
        esp.release()
        work.release()
        big.release()
        const.release()

    nc.finalize()
    return nc


def _get_nc():
    if "nc" not in _CACHE:
        _CACHE["nc"] = _build_nc()
    return _CACHE["nc"]


def kernel(**inputs):
    import ml_dtypes

    bf = ml_dtypes.bfloat16
    x = np.asarray(inputs["inputs"], dtype=np.float32)
    w_qk = np.asarray(inputs["W_qk"], dtype=np.float32)
    w_v = np.asarray(inputs["W_v"], dtype=np.float32)
    w_out = np.asarray(inputs["W_out"], dtype=np.float32) / 32.0

    # host-side transpose + supertile split: xts[b] is [SUP, D, 512] bf16
    xts = [
        np.ascontiguousarray(
            x[b].T.reshape(D, SUP, 512).transpose(1, 0, 2)).astype(bf)
        for b in range(B)
    ]
    nc = _get_nc()
    in_maps = []
    for c in range(N_CORES):
        b, g = divmod(c, 4)
        in_maps.append({
            "xt": xts[b],
            "w_qk": np.ascontiguousarray(
                w_qk[:, 512 * g:512 * (g + 1)]).astype(bf),
            "w_v": np.ascontiguousarray(
                w_v[:, 256 * g:256 * (g + 1)]).astype(bf),
            "w_out": np.ascontiguousarray(
                w_out[256 * g:256 * (g + 1), :]).astype(bf),
        })

    from concourse.bass_utils import run_bass_kernel_spmd

    trace = bool(os.environ.get("KERNEL_TRACE"))
    if trace:
        try:
            from antenv.axon_hooks import get_axon_ntff_profile_hook  # noqa: F401
        except Exception:
            trace = False
    res = run_bass_kernel_spmd(nc, in_maps, core_ids=list(range(N_CORES)),
                               trace=trace)
    _CACHE["last_results"] = res
    outs = [m["out"] for m in res.results]
    out = np.stack([
        outs[0] + outs[1] + outs[2] + outs[3],
        outs[4] + outs[5] + outs[6] + outs[7],
    ]).astype(np.float32)
    return out
